# revision 12
# baseline (speedup 1.0000x reference)
"""Contrastive loss kernel for Trainium2, 8 NeuronCores (SPMD row-sharded).

Computes: mean_i( -log( sum_j exp( (z/T) @ (z/T).T )_ij / N ) )
for z [16384, 128] fp32, T = 0.1.

Strategy per core (rows sharded 2048/core):
  - host: zs = z/T, transpose to [128, N], cast bf16 (PE runs bf16 at full
    rate; fp32 matmul is ~4x slower; the final scalar tolerates bf16 Gram
    inputs since PSUM accumulates fp32).
  - device: keep zs.T resident in SBUF. For each 128-row tile x 2048-col
    chunk: 4 matmuls [K=128]x[128,512] -> PSUM [128,2048] fp32; one
    ACTIVATE(Exp) over the chunk with accum_out giving per-partition row
    sums; DVE folds the 8 chunk partials. Output: rowsums [128, 16].
  - host: gather [16384] rowsums, l = -log(rowsum/N), mean -> scalar.
"""

import numpy as np
import ml_dtypes

TEMPERATURE = 0.1
N = 16384
D = 128
NCORES = 8
ROWS_PER_CORE = N // NCORES          # 2048
ROW_TILES = ROWS_PER_CORE // 128     # 16
CHUNK = 2048
NCHUNKS = N // CHUNK                 # 8
MM = 512                             # matmul moving free dim
MMS_PER_CHUNK = CHUNK // MM          # 4

_compiled = {}


def _build():
    import concourse.bacc as bacc
    import concourse.mybir as mybir
    import concourse.tile as tile

    bf16 = mybir.dt.bfloat16
    f32 = mybir.dt.float32

    # Bacc (not plain Bass): its finalize() runs the passes that move extra
    # matmul waits onto LDWEIGHTS / event semaphores -- the walrus MM
    # encoding only fits one sem wait.
    nc = bacc.Bacc()
    zT = nc.dram_tensor("zT", [D, N], bf16, kind="ExternalInput")
    zrT = nc.dram_tensor("zrT", [D, ROWS_PER_CORE], bf16, kind="ExternalInput")
    out = nc.dram_tensor("rowsums", [128, ROW_TILES], f32, kind="ExternalOutput")

    with tile.TileContext(nc) as tc:
        with (
            tc.tile_pool(name="persist", bufs=1) as persist,
            tc.tile_pool(name="work", bufs=4) as work,
            tc.tile_pool(name="psum", bufs=2, space="PSUM") as psum_pool,
        ):
            # Resident copy of zs.T; one tile per 2048-col chunk so matmuls
            # only wait on the chunk they read.
            zt_sb = []
            for cc in range(NCHUNKS):
                t = persist.tile([D, CHUNK], bf16, tag=f"zt{cc}")
                nc.sync.dma_start(out=t, in_=zT[:, cc * CHUNK:(cc + 1) * CHUNK])
                zt_sb.append(t)
            zr_sb = persist.tile([D, ROWS_PER_CORE], bf16, tag="zr")
            nc.sync.dma_start(out=zr_sb, in_=zrT[:, :])

            rsums = persist.tile([128, ROW_TILES], f32, tag="rsums")

            for rt in range(ROW_TILES):
                rparts = work.tile([128, NCHUNKS], f32, tag="rparts")
                for cc in range(NCHUNKS):
                    ps = psum_pool.tile([128, CHUNK], f32)
                    for m in range(MMS_PER_CHUNK):
                        nc.tensor.matmul(
                            ps[:, m * MM:(m + 1) * MM],
                            zr_sb[:, rt * 128:(rt + 1) * 128],
                            zt_sb[cc][:, m * MM:(m + 1) * MM],
                            start=True,
                            stop=True,
                        )
                    # exp over the chunk; accum_out = per-row partial sum.
                    # Main output goes to a write-only scratch (bf16 to halve
                    # the SBUF write traffic).
                    scratch = work.tile([128, CHUNK], bf16, tag="scratch")
                    nc.scalar.activation(
                        scratch,
                        ps,
                        mybir.ActivationFunctionType.Exp,
                        accum_out=rparts[:, cc:cc + 1],
                    )
                nc.vector.reduce_sum(
                    rsums[:, rt:rt + 1], rparts, axis=mybir.AxisListType.X
                )
            nc.sync.dma_start(out=out[:, :], in_=rsums)
    nc.finalize()
    return nc


def _get_nc():
    if "nc" not in _compiled:
        _compiled["nc"] = _build()
    return _compiled["nc"]


def kernel(z: np.ndarray) -> np.ndarray:
    from concourse.bass_utils import run_bass_kernel_spmd

    zs = np.asarray(z, dtype=np.float32) * np.float32(1.0 / TEMPERATURE)
    zsT = np.ascontiguousarray(zs.T).astype(ml_dtypes.bfloat16)  # [128, 16384]

    in_maps = []
    for c in range(NCORES):
        in_maps.append({
            "zT": zsT,
            "zrT": np.ascontiguousarray(
                zsT[:, c * ROWS_PER_CORE:(c + 1) * ROWS_PER_CORE]
            ),
        })

    nc = _get_nc()
    res = run_bass_kernel_spmd(nc, in_maps, list(range(NCORES)))

    # rowsums[p, rt] on core c is the row-sum for global row
    # c*2048 + rt*128 + p -> transpose each core's [128,16] and flatten.
    rowsums = np.concatenate(
        [np.asarray(r["rowsums"]).T.reshape(-1) for r in res.results]
    ).astype(np.float64)

    l = -(np.log(rowsums) - np.log(float(N)))
    return np.float32(l.mean())
